# revision 5
# baseline (speedup 1.0000x reference)
"""CZ gate (wires i=0, j=11) on a batch of 22-qubit statevectors.

The CZ gate is diagonal: it negates amplitude idx iff bit(n-1-i) and
bit(n-1-j) of idx are both 1.  For n=22, i=0, j=11 that is bit 21 and
bit 10.  Viewing each statevector as 4096 rows of 1024 floats, row r is
negated iff r >= 2048 (bit 21) and r is odd (bit 10 = LSB of r).

Sharding: pure data parallel — batch 8 across 8 NeuronCores, one full
statevector (16 MiB f32) per core.  Per core:
  - first half (8 MiB, contiguous): DRAM->DRAM DMA copy
  - even rows of second half (4 MiB, strided): DRAM->DRAM DMA copy
  - odd rows of second half (4 MiB): DMA->SBUF, negate on VectorE,
    DMA->DRAM, pipelined in chunks
"""

import sys

for _p in ("/opt/trn_rl_repo",):
    if _p not in sys.path:
        sys.path.insert(0, _p)

import numpy as np

import concourse.bass as bass
import concourse.mybir as mybir
import concourse.tile as tile
from concourse.bass_utils import run_bass_kernel_spmd

NQUBIT = 22
N = 1 << NQUBIT          # 4194304 floats per statevector
BATCH = 8
N_CORES = 8
ROW = 1024               # floats per "row" (= 2^10, set by j=11 -> bit 10)
HALF = N // 2

# Set by test harness to capture a profile; results land in LAST_RESULT.
TRACE = False
LAST_RESULT = None

_NC_CACHE = {}


def _build_nc(nchunk=2):
    """Raw-Bass kernel (no Tile): manual semaphores keep every instruction
    at <=1 sync wait (this walrus build rejects multi-wait instructions),
    and there is no Tile drain/barrier epilogue overhead.

    Engine plan:
      SP  (sync):   odd-row loads HBM->SBUF, then the 8 MiB first-half
                    DRAM->DRAM copy
      DVE (vector): negate odd rows SBUF->SBUF (out-of-place)
      ACT (scalar): 4 MiB even-row DRAM->DRAM copy, then odd-row stores
    """
    nc = bass.Bass()
    x = nc.dram_tensor("x", [N], mybir.dt.float32, kind="ExternalInput")
    y = nc.dram_tensor("y", [N], mybir.dt.float32, kind="ExternalOutput")

    # second half as (t, k, c): flat = HALF + (2k + t)*ROW + c
    xs = x[HALF:].rearrange("(k t c) -> t k c", t=2, c=ROW)
    ys = y[HALF:].rearrange("(k t c) -> t k c", t=2, c=ROW)
    n_odd = N // (2 * ROW) // 2          # 1024 odd rows
    rows_per_chunk = n_odd // nchunk
    shape = [128, rows_per_chunk // 128, ROW]

    import contextlib

    with contextlib.ExitStack() as ctx:
        tin = [
            ctx.enter_context(nc.sbuf_tensor(f"t{g}", shape, mybir.dt.float32))
            for g in range(nchunk)
        ]
        tout = [
            ctx.enter_context(nc.sbuf_tensor(f"o{g}", shape, mybir.dt.float32))
            for g in range(nchunk)
        ]
        ld = ctx.enter_context(nc.semaphore("ld"))
        st = ctx.enter_context(nc.semaphore("st"))
        ve = ctx.enter_context(nc.semaphore("ve"))
        block = ctx.enter_context(nc.Block())

        def src_dst(g):
            sl = slice(g * rows_per_chunk, (g + 1) * rows_per_chunk)
            src = xs[1][sl].rearrange("(p m) c -> p m c", p=128)
            dst = ys[1][sl].rearrange("(p m) c -> p m c", p=128)
            return src, dst

        @block.sync
        def _(sync):
            for g in range(nchunk):
                src, _ = src_dst(g)
                sync.dma_start(tin[g][:], src).then_inc(ld, 16)
            sync.dma_start(y[0:HALF], x[0:HALF]).then_inc(ld, 16)
            sync.wait_ge(ld, (nchunk + 1) * 16)

        @block.vector
        def _(vector):
            for g in range(nchunk):
                vector.wait_ge(ld, (g + 1) * 16)
                vector.tensor_scalar_mul(
                    tout[g][:].rearrange("p m c -> p (m c)"),
                    tin[g][:].rearrange("p m c -> p (m c)"),
                    -1.0,
                ).then_inc(ve, 1)

        @block.scalar
        def _(scalar):
            scalar.dma_start(ys[0], xs[0]).then_inc(st, 16)
            for g in range(nchunk):
                _, dst = src_dst(g)
                scalar.wait_ge(ve, g + 1)
                scalar.dma_start(dst, tout[g][:]).then_inc(st, 16)
            scalar.wait_ge(st, (nchunk + 1) * 16)

    return nc


def _numpy_fallback(x, i, j):
    n = int(round(np.log2(x.shape[1])))
    idx = np.arange(x.shape[1])
    mask = (((idx >> (n - 1 - i)) & 1) & ((idx >> (n - 1 - j)) & 1)).astype(bool)
    y = x.copy()
    y[:, mask] *= -1
    return y


def kernel(x, i, j):
    global LAST_RESULT
    x = np.ascontiguousarray(np.asarray(x, dtype=np.float32))
    i = int(np.asarray(i))
    j = int(np.asarray(j))
    if (i, j) != (0, 11) or x.shape != (BATCH, N):
        return _numpy_fallback(x, i, j)

    key = ("v1", TRACE)
    if key not in _NC_CACHE:
        _NC_CACHE[key] = _build_nc()
    nc = _NC_CACHE[key]

    in_maps = [{"x": x[c]} for c in range(N_CORES)]
    res = run_bass_kernel_spmd(
        nc, in_maps, core_ids=list(range(N_CORES)), trace=TRACE
    )
    LAST_RESULT = res
    return np.stack([r["y"] for r in res.results], axis=0)
